# revision 12
# baseline (speedup 1.0000x reference)
"""Trainium2 Bass kernel for nn_Actor (sampling): per-row Gaussian-grid
softmax + per-row gather, data-parallel over 32768 distribution rows on 8
NeuronCores.

Math: for each row r (r = lut*512 + j), mean m_r = tanh(x @ W + b)[r].
With grid g_i = -1 + 2i/127, std 0.5:
    probs[r, i] = exp(-0.5*((g_i - m_r)/0.5)^2 + c0),  c0 = -ln(0.5) - 0.5*ln(2pi)
    dist[r, i]  = exp(probs[r, i]) / S_r,   S_r = sum_i exp(probs[r, i])
    alp[e, r]   = dist[r, action[e, r]]

Key simplifications:
  * The gather is recomputed directly from the action index a:
    with z = s2*a + b_r (s2 = (2/127)/0.5, b_r = -2 - 2*m_r):
    alp = exp(exp(-0.5*z^2 + c0)) * invS_r  -- pure elementwise work.
    On device: zd = a + b_r/s2 (one tensor_tensor add against a 0-stride
    broadcast of the per-row vector), then Square applies the s2 scale.
  * S_r is an analytic, even function of m_r alone; 1/S is evaluated by a
    hard-coded degree-8 polynomial in u = m^2 (max rel err ~1.6e-9), so the
    128-point grid sweep is never materialized.
  * x @ W runs as bf16 hi/lo split-precision matmuls (x=xh+xl, W=Wh+Wl,
    dropping the lo*lo term; ~2^-17 relative error) directly in transposed
    layout, so the per-row mean lands with rows on partitions.

Per core: rows [c*4096, (c+1)*4096). The host pre-permutes actions into the
SBUF-resident layout [partition p, group g, epoch e] (value = action for row
g*128+p, as bf16 exact small ints) so every large DMA is fully dense, and
un-permutes the [128, 4096] f32 outputs.
"""

import math
import numpy as np
import ml_dtypes

N_CORES = 8
ROWS = 32768
R = ROWS // N_CORES          # 4096 rows per core
E = 128                      # epochs
G = R // 128                 # 32 row-groups of 128
FEAT = 256
M_OUT = 512
LUTS_PER_CORE = 8
X_DIM = 128
THR_STD = 0.5
X_RANGE = 1.0

S2 = float((2.0 * X_RANGE / (X_DIM - 1)) / THR_STD)
C0 = float(-math.log(THR_STD) - 0.5 * math.log(2.0 * math.pi))

# 1/S(m) as an even polynomial in u = m^2 (degree 8, rel err ~1.6e-9),
# S(m) = sum_i exp(exp(-0.5*((g_i - m)/std)^2 + c0)) over the fixed grid.
INVS_COEF = [
    0.004734368204497658,
    0.0006664708139734746,
    0.0005536234448627335,
    -3.841141713816861e-05,
    -8.543615513324016e-05,
    -3.230374091879953e-05,
    3.099837949298949e-05,
    -2.056325921731732e-06,
    -1.460328775896884e-06,
]

CH = 16                      # row-groups per compute chunk ([128, CH*128] tiles)
SQ_ON_DVE = (2,)             # global chunk indices whose Square runs on VectorE
ADD_ON_GP = (1, 3)           # global chunk indices whose z-add runs on GpSimd
MUL_ON_GP = (0, 2)           # global chunk indices whose invS-mult runs on GpSimd

_CACHE = {}


def _build():
    if "nc" in _CACHE:
        return _CACHE["nc"]

    from contextlib import ExitStack
    import concourse.bacc as bacc
    import concourse.mybir as mybir
    from concourse.tile import TileContext
    from concourse.alu_op_type import AluOpType

    dt = mybir.dt
    AF = mybir.ActivationFunctionType

    nc = bacc.Bacc("TRN2", target_bir_lowering=False, debug=False)

    # whl: [256, 1024] bf16 = [Wh | Wl];  xthl: [256, 16] bf16 = [xh^T | xl^T]
    # bias2: [128, 8] f32 = [bx by] as (jc, p) columns
    w_ext = [
        nc.declare_dram_parameter("whl_x", [FEAT, 2 * M_OUT], dt.bfloat16, isOutput=False),
        nc.declare_dram_parameter("whl_y", [FEAT, 2 * M_OUT], dt.bfloat16, isOutput=False),
    ]
    xt_ext = nc.declare_dram_parameter(
        "xthl", [FEAT, 2 * LUTS_PER_CORE], dt.bfloat16, isOutput=False
    )
    b_ext = nc.declare_dram_parameter("bias2", [128, 8], dt.float32, isOutput=False)
    a_ext = [
        nc.declare_dram_parameter("ax_t", [128, G * E], dt.bfloat16, isOutput=False),
        nc.declare_dram_parameter("ay_t", [128, G * E], dt.bfloat16, isOutput=False),
    ]
    o_ext = [
        nc.declare_dram_parameter("out_x", [128, G * E], dt.float32, isOutput=True),
        nc.declare_dram_parameter("out_y", [128, G * E], dt.float32, isOutput=True),
    ]

    with TileContext(nc) as tc, ExitStack() as ctx:
        const = ctx.enter_context(tc.tile_pool(name="const", bufs=1))
        psum = ctx.enter_context(tc.tile_pool(name="psum", bufs=2, space="PSUM"))
        zpool = ctx.enter_context(tc.tile_pool(name="zpool", bufs=2))
        z2pool = ctx.enter_context(tc.tile_pool(name="z2pool", bufs=2))
        ppool = ctx.enter_context(tc.tile_pool(name="ppool", bufs=2))
        epool = ctx.enter_context(tc.tile_pool(name="epool", bufs=2))
        opool = ctx.enter_context(tc.tile_pool(name="opool", bufs=3))

        # ---- parameter loads ------------------------------------------
        # small/means-critical tensors first on the sync HWDGE queue;
        # the two big action loads go on the scalar HWDGE queue so they
        # stream in parallel instead of delaying the means matmuls.
        xt_sb = const.tile([128, 2, 2 * LUTS_PER_CORE], dt.bfloat16, name="xthl")
        nc.sync.dma_start(
            out=xt_sb, in_=xt_ext.ap().rearrange("(kc p) i -> p kc i", p=128)
        )
        b_sb = const.tile([128, 8], dt.float32, name="bias2")
        nc.sync.dma_start(out=b_sb, in_=b_ext.ap())
        w_sb = []
        for d in range(2):
            w = const.tile([128, 2, 2 * M_OUT], dt.bfloat16, tag=f"w{d}")
            nc.sync.dma_start(
                out=w, in_=w_ext[d].ap().rearrange("(kc p) j -> p kc j", p=128)
            )
            w_sb.append(w)
        c0_bias = const.tile([128, 1], dt.float32, name="c0_bias")
        nc.vector.memset(c0_bias, C0)
        a_sb = []
        half = G * E // 2
        for d in range(2):
            a = const.tile([128, G * E], dt.bfloat16, tag=f"a{d}")
            nc.sync.dma_start(out=a[:, 0:half], in_=a_ext[d].ap()[:, 0:half])
            nc.sync.dma_start(out=a[:, half:], in_=a_ext[d].ap()[:, half:])
            a_sb.append(a)

        # ---- per-dist means directly in meanT layout ------------------
        # psum[j', i] = sum_k W[k, jc*128+j'] * x[i, k] via 3 bf16 hi/lo
        # products per K-chunk; tanh(. + b) with per-partition bias.
        def means(d):
            m_d = const.tile([128, G], dt.float32, tag=f"m{d}")
            for jc in range(4):
                pt = psum.tile([128, LUTS_PER_CORE], dt.float32)
                first = True
                for kc in range(2):
                    wh = w_sb[d][:, kc, jc * 128:(jc + 1) * 128]
                    wl = w_sb[d][:, kc, M_OUT + jc * 128:M_OUT + (jc + 1) * 128]
                    xh = xt_sb[:, kc, 0:LUTS_PER_CORE]
                    xl = xt_sb[:, kc, LUTS_PER_CORE:2 * LUTS_PER_CORE]
                    nc.tensor.matmul(pt, lhsT=wh, rhs=xh, start=first, stop=False)
                    nc.tensor.matmul(pt, lhsT=wh, rhs=xl, start=False, stop=False)
                    nc.tensor.matmul(
                        pt, lhsT=wl, rhs=xh,
                        start=False, stop=(kc == 1),
                    )
                    first = False
                # group g = i*4 + jc  ->  m_d[:, jc::4]
                nc.scalar.activation(
                    m_d[:, jc:G:4], pt, AF.Tanh,
                    bias=b_sb[:, d * 4 + jc:d * 4 + jc + 1],
                )
            return m_d

        def prep(d, m_d):
            # bdiv = b_r / s2 = (-2 - 2 m)/s2 ; invS = poly(m^2)
            u = const.tile([128, G], dt.float32, tag=f"u{d}")
            nc.vector.tensor_tensor(u, m_d, m_d, AluOpType.mult)
            bdiv = const.tile([128, G], dt.float32, tag=f"bdiv{d}")
            nc.vector.tensor_scalar(
                bdiv, m_d, -2.0 / S2, -2.0 / S2, AluOpType.mult, AluOpType.add
            )
            h = const.tile([128, G], dt.float32, tag=f"h{d}")
            t = const.tile([128, G], dt.float32, tag=f"t{d}")
            nc.vector.tensor_scalar(
                h, u, float(INVS_COEF[8]), float(INVS_COEF[7]),
                AluOpType.mult, AluOpType.add,
            )
            for k in range(6, -1, -1):
                nc.vector.tensor_tensor(t, h, u, AluOpType.mult)
                nc.vector.tensor_scalar(
                    h, t, float(INVS_COEF[k]), None, AluOpType.add
                )
            return bdiv, h

        def main_loop(d, bdiv, invs):
            a_re = a_sb[d][:, :].rearrange("p (g e) -> p g e", e=E)
            n_chunks = G // CH
            for c2 in range(n_chunks):
                g0 = c2 * CH
                bd_b = bdiv[:, g0:g0 + CH].to_broadcast([128, CH, E])
                iv_b = invs[:, g0:g0 + CH].to_broadcast([128, CH, E])
                z = zpool.tile([128, CH, E], dt.float32)
                add_eng = nc.gpsimd if (c2 + 2 * d) in ADD_ON_GP else nc.vector
                add_eng.tensor_tensor(
                    z, a_re[:, g0:g0 + CH, :], bd_b, AluOpType.add
                )
                z2 = z2pool.tile([128, CH, E], dt.float32)
                if c2 + 2 * d in SQ_ON_DVE:
                    nc.vector.tensor_tensor(z2, z, z, AluOpType.mult)
                    exp_scale = -0.5 * S2 * S2
                else:
                    nc.scalar.activation(z2, z, AF.Square, scale=S2)
                    exp_scale = -0.5
                p = ppool.tile([128, CH, E], dt.float32)
                nc.scalar.activation(p, z2, AF.Exp, bias=c0_bias, scale=exp_scale)
                e = epool.tile([128, CH, E], dt.float32)
                nc.scalar.activation(e, p, AF.Exp)
                o = opool.tile([128, CH, E], dt.float32)
                mul_eng = nc.gpsimd if (c2 + 2 * d) in MUL_ON_GP else nc.vector
                mul_eng.tensor_tensor(o, e, iv_b, AluOpType.mult)
                nc.sync.dma_start(
                    out=o_ext[d].ap()[:, g0 * E:(g0 + CH) * E], in_=o
                )

        m0 = means(0)
        bdiv0, invs0 = prep(0, m0)
        m1 = means(1)
        bdiv1, invs1 = prep(1, m1)
        main_loop(0, bdiv0, invs0)
        main_loop(1, bdiv1, invs1)

    nc.compile()
    _CACHE["nc"] = nc
    return nc


LAST_RESULTS = None


def _to_device_layout(action):
    # [E, ROWS] int -> per-core [128, G*E] bf16 with value(p, g, e) =
    # action[e, core_base + g*128 + p]
    a = np.asarray(action).T.astype(np.float32).astype(ml_dtypes.bfloat16)
    a = a.reshape(N_CORES, G, 128, E)          # [core, g, p, e]
    a = a.transpose(0, 2, 1, 3)                # [core, p, g, e]
    return np.ascontiguousarray(a.reshape(N_CORES, 128, G * E))


def _hilo(v):
    hi = v.astype(ml_dtypes.bfloat16)
    lo = (v - hi.astype(np.float32)).astype(ml_dtypes.bfloat16)
    return hi, lo


def kernel(x, Wx, bx, Wy, by, action_x, action_y):
    global LAST_RESULTS
    from concourse.bass_utils import run_bass_kernel_spmd

    nc = _build()

    x = np.asarray(x, dtype=np.float32)
    whl = []
    for W in (Wx, Wy):
        wh, wl = _hilo(np.asarray(W, dtype=np.float32))
        whl.append(np.ascontiguousarray(np.concatenate([wh, wl], axis=1)))
    bias2 = np.ascontiguousarray(
        np.concatenate(
            [np.asarray(bx, np.float32).reshape(4, 128).T,
             np.asarray(by, np.float32).reshape(4, 128).T], axis=1)
    )
    ax_d = _to_device_layout(action_x)
    ay_d = _to_device_layout(action_y)

    in_maps = []
    for c in range(N_CORES):
        xc = x[c * LUTS_PER_CORE:(c + 1) * LUTS_PER_CORE].T   # [256, 8]
        xh, xl = _hilo(np.ascontiguousarray(xc))
        xthl = np.ascontiguousarray(np.concatenate([xh, xl], axis=1))
        in_maps.append({
            "ax_t": ax_d[c],
            "ay_t": ay_d[c],
            "whl_x": whl[0],
            "whl_y": whl[1],
            "bias2": bias2,
            "xthl": xthl,
        })

    res = run_bass_kernel_spmd(nc, in_maps, core_ids=list(range(N_CORES)))
    LAST_RESULTS = res

    alp_x = np.empty((E, ROWS), dtype=np.float32)
    alp_y = np.empty((E, ROWS), dtype=np.float32)
    for c in range(N_CORES):
        for name, alp in (("out_x", alp_x), ("out_y", alp_y)):
            o = res.results[c][name].reshape(128, G, E)   # [p, g, e]
            o = o.transpose(2, 1, 0).reshape(E, R)        # [e, (g p)]
            alp[:, c * R:(c + 1) * R] = o
    ent = np.float32(0.5 + 0.5 * math.log(2.0 * math.pi) + math.log(THR_STD))
    return alp_x, alp_y, ent, ent


# revision 27
# speedup vs baseline: 1.1707x; 1.1707x over previous
"""Trainium2 Bass kernel for nn_Actor (sampling): per-row Gaussian-grid
softmax + per-row gather, data-parallel over 32768 distribution rows on 8
NeuronCores.

Math: for each row r (r = lut*512 + j), mean m_r = tanh(x @ W + b)[r].
With grid g_i = -1 + 2i/127, std 0.5:
    probs[r, i] = exp(-0.5*((g_i - m_r)/0.5)^2 + c0),  c0 = -ln(0.5) - 0.5*ln(2pi)
    dist[r, i]  = exp(probs[r, i]) / S_r,   S_r = sum_i exp(probs[r, i])
    alp[e, r]   = dist[r, action[e, r]]

Key simplifications:
  * The gather is recomputed directly from the action index a:
    with z = s2*a + b_r (s2 = (2/127)/0.5, b_r = -2 - 2*m_r):
    alp = exp(exp(-0.5*z^2 + c0)) * invS_r  -- pure elementwise work.
    On device: zd = a + b_r/s2 via one tensor_tensor add against a 0-stride
    broadcast of the per-row vector; the s2 scale folds into later stages.
  * S_r is an analytic, even function of m_r alone; 1/S is evaluated by a
    hard-coded degree-8 polynomial in u = m^2 (max rel err ~1.6e-9), so the
    128-point grid sweep is never materialized.
  * x @ W runs as bf16 hi/lo split-precision matmuls (x=xh+xl, W=Wh+Wl,
    dropping the lo*lo term; ~2^-17 relative error) directly in transposed
    layout, so the per-row mean lands with rows on partitions.

Per core: rows [c*4096, (c+1)*4096). The host pre-permutes actions into the
SBUF-resident layout [partition p, group g, epoch e] (value = action for row
g*128+p, as bf16 exact small ints) so every large DMA is fully dense, and
un-permutes the [128, 4096] outputs (bf16 on the wire, upcast on host).
"""

import math
import numpy as np
import ml_dtypes

N_CORES = 8
ROWS = 32768
R = ROWS // N_CORES          # 4096 rows per core
E = 128                      # epochs
G = R // 128                 # 32 row-groups of 128
FEAT = 256
M_OUT = 512
LUTS_PER_CORE = 8
X_DIM = 128
THR_STD = 0.5
X_RANGE = 1.0

S2 = float((2.0 * X_RANGE / (X_DIM - 1)) / THR_STD)
C0 = float(-math.log(THR_STD) - 0.5 * math.log(2.0 * math.pi))

# 1/S(m) as an even polynomial in u = m^2 (degree 8, rel err ~1.6e-9),
# S(m) = sum_i exp(exp(-0.5*((g_i - m)/std)^2 + c0)) over the fixed grid.
INVS_COEF = [
    0.004734368204497658,
    0.0006664708139734746,
    0.0005536234448627335,
    -3.841141713816861e-05,
    -8.543615513324016e-05,
    -3.230374091879953e-05,
    3.099837949298949e-05,
    -2.056325921731732e-06,
    -1.460328775896884e-06,
]

CH = 16                      # row-groups per compute chunk ([128, CH*128] tiles)
SQ_ON_DVE = (1, 3)             # global chunk indices whose Square runs on VectorE
OUT_BF16 = True              # bf16 on-the-wire outputs (upcast on host)

_CACHE = {}


def _build():
    if "nc" in _CACHE:
        return _CACHE["nc"]

    from contextlib import ExitStack
    import concourse.bacc as bacc
    import concourse.mybir as mybir
    from concourse.tile import TileContext
    from concourse.alu_op_type import AluOpType

    dt = mybir.dt
    AF = mybir.ActivationFunctionType
    L = LUTS_PER_CORE
    odt = dt.bfloat16 if OUT_BF16 else dt.float32

    nc = bacc.Bacc("TRN2", target_bir_lowering=False, debug=False)

    # whl: [256, 1024] bf16 = [Wh | Wl];  xthl: [256, 16] bf16 = [xh^T | xl^T]
    # bias2: [128, 8] f32 = [bx by] as (jc, p) columns
    w_ext = [
        nc.declare_dram_parameter("whl_x", [FEAT, 2 * M_OUT], dt.bfloat16, isOutput=False),
        nc.declare_dram_parameter("whl_y", [FEAT, 2 * M_OUT], dt.bfloat16, isOutput=False),
    ]
    xt_ext = nc.declare_dram_parameter("xthl", [FEAT, 2 * L], dt.bfloat16, isOutput=False)
    b_ext = nc.declare_dram_parameter("bias2", [128, 8], dt.float32, isOutput=False)
    a_ext = [
        nc.declare_dram_parameter("ax_t", [128, G * E], dt.bfloat16, isOutput=False),
        nc.declare_dram_parameter("ay_t", [128, G * E], dt.bfloat16, isOutput=False),
    ]
    o_ext = [
        nc.declare_dram_parameter("out_x", [128, G * E], odt, isOutput=True),
        nc.declare_dram_parameter("out_y", [128, G * E], odt, isOutput=True),
    ]

    with TileContext(nc) as tc, ExitStack() as ctx:
        const = ctx.enter_context(tc.tile_pool(name="const", bufs=1))
        psum = ctx.enter_context(tc.tile_pool(name="psum", bufs=2, space="PSUM"))
        zpool = ctx.enter_context(tc.tile_pool(name="zpool", bufs=2))
        z2pool = ctx.enter_context(tc.tile_pool(name="z2pool", bufs=2))
        ppool = ctx.enter_context(tc.tile_pool(name="ppool", bufs=2))
        epool = ctx.enter_context(tc.tile_pool(name="epool", bufs=2))
        opool = ctx.enter_context(tc.tile_pool(name="opool", bufs=3))

        # ---- parameter loads ------------------------------------------
        w_sb = []
        for d in range(2):
            w = const.tile([128, 2, 2 * M_OUT], dt.bfloat16, tag=f"w{d}")
            nc.sync.dma_start(
                out=w, in_=w_ext[d].ap().rearrange("(kc p) j -> p kc j", p=128)
            )
            w_sb.append(w)
        xt_sb = const.tile([128, 2, 2 * L], dt.bfloat16, name="xthl")
        nc.sync.dma_start(
            out=xt_sb, in_=xt_ext.ap().rearrange("(kc p) i -> p kc i", p=128)
        )
        b_sb = const.tile([128, 8], dt.float32, name="bias2")
        nc.sync.dma_start(out=b_sb, in_=b_ext.ap())
        c0_bias = const.tile([128, 1], dt.float32, name="c0_bias")
        nc.vector.memset(c0_bias, C0)
        a_sb = []
        for d in range(2):
            a = const.tile([128, G * E], dt.bfloat16, tag=f"a{d}")
            nc.sync.dma_start(out=a, in_=a_ext[d].ap())
            a_sb.append(a)

        # ---- per-dist means directly in meanT layout ------------------
        def means(d):
            m_d = const.tile([128, G], dt.float32, tag=f"m{d}")
            for jc in range(4):
                pt = psum.tile([128, L], dt.float32, tag="pt")
                first = True
                for kc in range(2):
                    wh = w_sb[d][:, kc, jc * 128:(jc + 1) * 128]
                    wl = w_sb[d][:, kc, M_OUT + jc * 128:M_OUT + (jc + 1) * 128]
                    xh = xt_sb[:, kc, 0:L]
                    xl = xt_sb[:, kc, L:2 * L]
                    nc.tensor.matmul(pt, lhsT=wh, rhs=xh, start=first, stop=False)
                    nc.tensor.matmul(pt, lhsT=wh, rhs=xl, start=False, stop=False)
                    nc.tensor.matmul(
                        pt, lhsT=wl, rhs=xh, start=False, stop=(kc == 1)
                    )
                    first = False
                # group g = i*4 + jc  ->  m_d[:, jc::4]
                nc.scalar.activation(
                    m_d[:, jc:G:4], pt, AF.Tanh,
                    bias=b_sb[:, d * 4 + jc:d * 4 + jc + 1],
                )
            return m_d

        def prep(d, m_d):
            # bdiv = b_r / s2 = (-2 - 2 m)/s2 ; invS = poly(m^2)
            u = const.tile([128, G], dt.float32, tag=f"u{d}")
            nc.vector.tensor_tensor(u, m_d, m_d, AluOpType.mult)
            bdiv = const.tile([128, G], dt.float32, tag=f"bdiv{d}")
            nc.vector.tensor_scalar(
                bdiv, m_d, -2.0 / S2, -2.0 / S2, AluOpType.mult, AluOpType.add
            )
            h = const.tile([128, G], dt.float32, tag=f"h{d}")
            t = const.tile([128, G], dt.float32, tag=f"t{d}")
            nc.vector.tensor_scalar(
                h, u, float(INVS_COEF[8]), float(INVS_COEF[7]),
                AluOpType.mult, AluOpType.add,
            )
            for k in range(6, -1, -1):
                nc.vector.tensor_tensor(t, h, u, AluOpType.mult)
                nc.vector.tensor_scalar(
                    h, t, float(INVS_COEF[k]), None, AluOpType.add
                )
            return bdiv, h

        def main_loop(d, bdiv, invs):
            a_re = a_sb[d][:, :].rearrange("p (g e) -> p g e", e=E)
            n_chunks = G // CH
            for c2 in range(n_chunks):
                g0 = c2 * CH
                bd_b = bdiv[:, g0:g0 + CH].to_broadcast([128, CH, E])
                iv_b = invs[:, g0:g0 + CH].to_broadcast([128, CH, E])
                z = zpool.tile([128, CH, E], dt.float32)
                nc.vector.tensor_tensor(
                    z, a_re[:, g0:g0 + CH, :], bd_b, AluOpType.add
                )
                z2 = z2pool.tile([128, CH, E], dt.float32)
                if c2 + 2 * d in SQ_ON_DVE:
                    nc.vector.tensor_tensor(z2, z, z, AluOpType.mult)
                    exp_scale = -0.5 * S2 * S2
                else:
                    nc.scalar.activation(z2, z, AF.Square, scale=S2)
                    exp_scale = -0.5
                p = ppool.tile([128, CH, E], dt.float32)
                nc.scalar.activation(p, z2, AF.Exp, bias=c0_bias, scale=exp_scale)
                e = epool.tile([128, CH, E], dt.float32)
                nc.scalar.activation(e, p, AF.Exp)
                o = opool.tile([128, CH, E], odt)
                nc.vector.tensor_tensor(o, e, iv_b, AluOpType.mult)
                half = CH // 2
                nc.sync.dma_start(
                    out=o_ext[d].ap()[:, g0 * E:(g0 + half) * E],
                    in_=o[:, 0:half, :],
                )
                nc.sync.dma_start(
                    out=o_ext[d].ap()[:, (g0 + half) * E:(g0 + CH) * E],
                    in_=o[:, half:CH, :],
                )

        m0 = means(0)
        bdiv0, invs0 = prep(0, m0)
        m1 = means(1)
        bdiv1, invs1 = prep(1, m1)
        main_loop(0, bdiv0, invs0)
        main_loop(1, bdiv1, invs1)

    nc.compile()
    _CACHE["nc"] = nc
    return nc


LAST_RESULTS = None


def _to_device_layout(action):
    # [E, ROWS] int -> per-core [128, G*E] bf16 with value(p, g, e) =
    # action[e, core_base + g*128 + p]
    a = np.asarray(action).T.astype(np.float32).astype(ml_dtypes.bfloat16)
    a = a.reshape(N_CORES, G, 128, E)          # [core, g, p, e]
    a = a.transpose(0, 2, 1, 3)                # [core, p, g, e]
    return np.ascontiguousarray(a.reshape(N_CORES, 128, G * E))


def _hilo(v):
    hi = v.astype(ml_dtypes.bfloat16)
    lo = (v - hi.astype(np.float32)).astype(ml_dtypes.bfloat16)
    return hi, lo


def kernel(x, Wx, bx, Wy, by, action_x, action_y):
    global LAST_RESULTS
    from concourse.bass_utils import run_bass_kernel_spmd

    nc = _build()
    L = LUTS_PER_CORE

    x = np.asarray(x, dtype=np.float32)
    whl = []
    for W in (Wx, Wy):
        wh, wl = _hilo(np.asarray(W, dtype=np.float32))
        whl.append(np.ascontiguousarray(np.concatenate([wh, wl], axis=1)))
    bias2 = np.ascontiguousarray(
        np.concatenate(
            [np.asarray(bx, np.float32).reshape(4, 128).T,
             np.asarray(by, np.float32).reshape(4, 128).T], axis=1)
    )
    ax_d = _to_device_layout(action_x)
    ay_d = _to_device_layout(action_y)

    in_maps = []
    for c in range(N_CORES):
        xc = x[c * L:(c + 1) * L].T                 # [256, 8]
        xh, xl = _hilo(np.ascontiguousarray(xc))
        xthl = np.ascontiguousarray(np.concatenate([xh, xl], axis=1))
        in_maps.append({
            "ax_t": ax_d[c],
            "ay_t": ay_d[c],
            "whl_x": whl[0],
            "whl_y": whl[1],
            "bias2": bias2,
            "xthl": xthl,
        })

    res = run_bass_kernel_spmd(nc, in_maps, core_ids=list(range(N_CORES)))
    LAST_RESULTS = res

    alp_x = np.empty((E, ROWS), dtype=np.float32)
    alp_y = np.empty((E, ROWS), dtype=np.float32)
    for c in range(N_CORES):
        for name, alp in (("out_x", alp_x), ("out_y", alp_y)):
            o = res.results[c][name].astype(np.float32).reshape(128, G, E)
            o = o.transpose(2, 1, 0).reshape(E, R)        # [e, (g p)]
            alp[:, c * R:(c + 1) * R] = o
    ent = np.float32(0.5 + 0.5 * math.log(2.0 * math.pi) + math.log(THR_STD))
    return alp_x, alp_y, ent, ent
